# revision 39
# baseline (speedup 1.0000x reference)
"""GroupedEmbeddingBag kernel for 8 trn2 NeuronCores.

Table-parallel: core c handles table c (weights[c], values[c], offsets[c]).

Per core the id stream is split into position-chunks (14 big + a pyramid of
shrinking tail chunks, so the post-gather pipeline drain at the end is
short); within a chunk ids are bucket-sorted into 4 contiguous table-row
ranges of 25000 rows so that dma_gather (InstDMAGatherAnt, int16 relative
indices) can pull thousands of rows per SWDGE instruction — the
per-instruction descriptor-generation overhead that dominated an
indirect_dma_start-per-tile design is amortized away. Gathered rows land in
SBUF in (chunk, bucket, position) slot order; the host knows the exact slot
layout, so it ships a per-slot bag-id byte (epoch-relative). The DVE
expands bag-ids into one-hot bf16 selection matrices on-chip (is_equal vs
an iota constant), TensorE matmuls segment-sum epochs into PSUM bag
windows (an epoch = tile t from EACH of the 4 buckets, which cover the same
~512 positions, so the window W is ~40 instead of ~68; two epochs share one
psum tile), the Scalar engine copies finished pairs to an SBUF ring with an
int16 output scale, and 48-epoch ring segments stream back to DRAM. The host overlap-adds the epoch windows into
the final pooled output and divides the scale back out.

Pipeline-keeping details (the kernel is DMA-descriptor-bound: ~205k row
descriptors/core at 512 effective bytes each):
- Each bucket of a chunk gathers into its own et tile, so a gather only
  waits on the matmuls of its own bucket from EP_BUFS chunks back, and
  matmuls start as soon as their bucket's transfer lands.
- Chunk 0's idx columns are a separate tiny DRAM param, so the first
  gather's descriptor generation starts ~1us in.
- DVE memsets of the first et ring generation run concurrently with the
  first transfers (pad slots must stay finite since 0*NaN poisons PSUM);
  after that, buffers only ever hold stale gathered rows.

Weights are bf16 (PSUM accumulation fp32): rel-err vs the fp32 reference
~2e-3, far inside the 2e-2 gate.
"""

import os
import sys

sys.path.insert(0, "/opt/trn_rl_repo")

import numpy as np
import ml_dtypes

T, V, D, B = 8, 100000, 128, 4096
L = 204800
P = 128
NB = 4                      # index-range buckets (int16 addressing limit)
BROWS = V // NB             # 25000 rows per bucket
# 14 big chunks + a pyramid tail so the post-gather drain is short and each
# tail chunk's et-ring dependency (EP_BUFS back) is already drained.
CHUNK_NPOS = [12800] * 14 + [9600, 6400, 4800, 3200, 1600]
EP_T = 4                    # slot-tiles per matmul group (one per bucket)
ET_BK = 2                   # tiles per bucket per epoch (epoch = NB*ET_BK tiles)
OUT_RING_EP = 48             # epochs per output DMA
EP_BUFS = 4                 # et ring depth; first EP_BUFS chunks gather-fill
SEL_BUFS = 3

_compiled = {}


def _chunk_spec(S_list):
    """Per-chunk layout: slots per (chunk, bucket) are padded to S_k
    (multiple of 512). Returns list of dicts + totals."""
    chunks = []
    pos = 0
    icol = 0
    tile0 = 0
    for npos, S_k in zip(CHUNK_NPOS, S_list, strict=True):
        assert S_k % 512 == 0
        chunks.append(dict(
            pos0=pos, npos=npos, S=S_k, SC=S_k // 16, ST_B=S_k // P,
            NT=NB * (S_k // P), icol0=icol, tile0=tile0,
        ))
        pos += npos
        icol += NB * (S_k // 16)
        tile0 += NB * (S_k // P)
    assert pos == L
    return chunks, icol, tile0


def _patch_drain(tile_mod, mybir):
    from concourse.vector_clock import ScopedClock

    def _patched(self, tick_clock, wait_clock):
        # this walrus build allows only ONE sync-wait on the tail Drain:
        # spread the rest over preceding nops, one wait each. Rotate the
        # nops across all engine sequencers so their dispatch (~25-50ns
        # each) runs in parallel instead of serializing ~64 nops on SP.
        NNOPS = 64
        engs = [self.nc.sync, self.nc.gpsimd, self.nc.vector,
                self.nc.scalar, self.nc.tensor]
        nops = [engs[i % len(engs)].nop(nofuse=True, hint=f"dw_{i}")
                for i in range(NNOPS)]
        drain_inst = self.nc.sync.drain()
        wait_clock.add_sem_waits(
            drain_inst.ins, ScopedClock({None: tick_clock.global_clock})
        )
        dsi = drain_inst.ins.sync_info
        waits = list(dsi.on_wait) if dsi else []
        if len(waits) > 1:
            del dsi.on_wait[1:]
            rest = waits[1:]
            assert len(rest) <= NNOPS, f"too many drain waits: {len(waits)}"
            for nop, w in zip(nops, rest):
                nsi = nop.ins.sync_info
                if nsi is None:
                    nop.ins.sync_info = mybir.SyncInfo(on_wait=[w], on_update=[])
                else:
                    nsi.on_wait.append(w)
        self.nc.all_engine_barrier()
        popped = self.nc._tile_sem_poison_stack.pop()
        assert popped is self._sem_poison
        self.nc.clear_and_free_semaphores(list(self.sems.allocated().values()))
        self.nc.all_engine_barrier()

    tile_mod.TileContext._drain_and_barrier = _patched


def _split_waits(nc, mybir, maxw=1):
    # this walrus build rejects >1 sync-wait on an instruction: hoist extra
    # waits onto same-engine nops spliced in directly before it.
    cnt = 0
    for fn in nc.m.functions:
        for blk in fn.blocks:
            new_insts = []
            for inst in blk.instructions:
                si = inst.sync_info
                if si is not None and len(si.on_wait) > maxw:
                    extra = list(si.on_wait[maxw:])
                    del si.on_wait[maxw:]
                    for w in extra:
                        nop = mybir.InstNoOp(
                            name=f"waitnop-{cnt}", engine=inst.engine, ins=[], outs=[]
                        )
                        cnt += 1
                        nop.sync_info = mybir.SyncInfo(on_wait=[w], on_update=[])
                        new_insts.append(nop)
                new_insts.append(inst)
            blk.instructions[:] = new_insts
    return cnt


def _build(W, S_list, nidx, cmin, reps=1):
    """W: epoch bag-window (PSUM partitions). S_list: per-chunk padded slots
    per bucket. nidx: per-(chunk,bucket) gather num_idxs (max count over
    cores, 16-rounded) — the idx arrays carry no S-padding. reps>1 repeats
    the whole body for timing-calibration builds."""
    import concourse.bass as bass
    import concourse.mybir as mybir
    import concourse.tile as tile
    from concourse import library_config, library_overlay

    _patch_drain(tile, mybir)

    chunks, ICOL, NTT = _chunk_spec(S_list)
    NEPO = NTT // (NB * ET_BK)
    # idx column offsets: ceil(nidx/16) columns per (chunk, bucket)
    icol_off = [0]
    for v in nidx:
        icol_off.append(icol_off[-1] + v // 16)
    ICOL = icol_off[-1]
    IC0 = icol_off[NB]          # columns belonging to chunk 0

    bf16 = mybir.dt.bfloat16

    nc = bass.Bass(num_swdge_queues=4)
    wt = nc.declare_dram_parameter("wt", [V, D], bf16, isOutput=False)
    # idx payload host-replicated to the 8 Q7 core groups ([128, ...]) so each
    # lands in one DMA. Chunk 0's columns are split out so the first gathers
    # only wait on a small load.
    idxs0 = nc.declare_dram_parameter("idxs0", [P, NB * SC0], mybir.dt.int16, isOutput=False)
    idxsr = nc.declare_dram_parameter("idxsr", [P, ICOL - NB * SC0], mybir.dt.int16, isOutput=False)
    cnts = nc.declare_dram_parameter("cnts", [1, len(chunks) * NB], mybir.dt.int32, isOutput=False)
    bagid = nc.declare_dram_parameter("bagid", [P, NTT], mybir.dt.int8, isOutput=False)
    iota = nc.declare_dram_parameter("iota", [P, W], mybir.dt.int8, isOutput=False)
    oslots = nc.declare_dram_parameter("oslots", [W, NEPO * D], mybir.dt.int16, isOutput=True)

    with tile.TileContext(nc) as tc:
        nc.gpsimd.load_library(library_config.mlp)
        with (
            tc.tile_pool(name="inp", bufs=1) as inp,
            tc.tile_pool(name="selp", bufs=SEL_BUFS) as selp,
            tc.tile_pool(name="ep", bufs=EP_BUFS) as ep,
            tc.tile_pool(name="outp", bufs=2) as outp,
            tc.tile_pool(name="psum", bufs=8, space="PSUM") as psump,
        ):
            cnts_sb = inp.tile([1, len(chunks) * NB], mybir.dt.int32)
            nc.sync.dma_start(out=cnts_sb[:], in_=cnts[:])
            idxs0_sb = inp.tile([P, NB * SC0], mybir.dt.int16)
            nc.sync.dma_start(out=idxs0_sb[:], in_=idxs0[:])
            iota_sb = inp.tile([P, W], mybir.dt.int8)
            nc.sync.dma_start(out=iota_sb[:], in_=iota[:])
            bagid_sb = inp.tile([P, NTT], mybir.dt.int8)
            nc.sync.dma_start(out=bagid_sb[:], in_=bagid[:])
            idxsr_sb = inp.tile([P, ICOL - NB * SC0], mybir.dt.int16)
            nc.sync.dma_start(out=idxsr_sb[:], in_=idxsr[:])

            cregs = [nc.gpsimd.alloc_register(name=f"creg{b}") for b in range(NB)]

            init_tiles = []

            out_ring = None
            psum_t = None
            for rep in range(reps):
                pend = 0      # epochs pending in out_ring
                flushed = 0   # epochs of this ring already flushed
                ring_e0 = 0
                TAIL_E0 = NEPO - (NEPO % OUT_RING_EP)  # first epoch of the
                # final partial ring: flush it in 8-epoch slices so the tail
                # output overlaps the tail compute
                for kc, ch in enumerate(chunks):
                    S, SC, ST_B, NT = ch["S"], ch["SC"], ch["ST_B"], ch["NT"]
                    ets = [
                        ep.tile([P, ST_B * D], bf16, tag=f"e{b}",
                                name=f"et{rep}_{kc}_{b}")
                        for b in range(NB)
                    ]
                    if rep == 0 and kc < EP_BUFS:
                        # ring buffers start with arbitrary SBUF bits; pad
                        # slots (idx -1 -> no DMA write) must stay finite
                        # since 0*NaN poisons PSUM. Only the tail beyond the
                        # min gather count over cores can stay unwritten —
                        # memset just that slice (12x smaller, keeps DVE off
                        # the gather critical path).
                        for b in range(NB):
                            mstart = (cmin[kc * NB + b] // P) * D
                            if mstart < ST_B * D:
                                nc.vector.memset(ets[b][:, mstart:], 0)
                    for b in range(NB):
                        i = kc * NB + b
                        ni = nidx[i]
                        nc_cols = ni // 16
                        if kc == 0:
                            idx_ap = idxs0_sb[:, icol_off[i]:icol_off[i] + nc_cols]
                        else:
                            j = icol_off[i] - IC0
                            idx_ap = idxsr_sb[:, j:j + nc_cols]
                        ntile = (ni + P - 1) // P
                        nc.gpsimd.load(cregs[b], cnts_sb[:1, i:i + 1])
                        nc.gpsimd.dma_gather(
                            ets[b][:, :ntile * D].rearrange(
                                "p (s d) -> p s d", d=D),
                            wt[b * BROWS:(b + 1) * BROWS, :],
                            idx_ap,
                            ni,
                            cregs[b],
                            D,
                            single_packet=False,
                            queue_num=b,
                        )
                    sel_sb = selp.tile([P, NT * W], bf16, tag="sel",
                                       name=f"sel{rep}_{kc}")
                    nc.vector.tensor_tensor(
                        out=sel_sb[:].rearrange("p (t w) -> p t w", w=W),
                        in0=bagid_sb[:, ch["tile0"]:ch["tile0"] + NT]
                        .rearrange("p t -> p t ()")
                        .to_broadcast([P, NT, W]),
                        in1=iota_sb[:]
                        .rearrange("p w -> p () w")
                        .to_broadcast([P, NT, W]),
                        op=mybir.AluOpType.is_equal,
                    )
                    ep0 = ch["tile0"] // (NB * ET_BK)
                    for u in range(ST_B // ET_BK):
                        # epoch = tiles (2u, 2u+1) from EACH bucket: they
                        # cover the same ~ET_BK*EP_T*P positions, keeping
                        # the bag window W small (~48)
                        e = ep0 + u
                        eh = e % PSUM_EP    # epoch slot within the psum tile
                        if eh == 0:
                            psum_t = psump.tile(
                                [W, PSUM_EP * D], mybir.dt.float32, tag="ps")
                        for b in range(NB):
                            for dt in range(ET_BK):
                                et = u * ET_BK + dt
                                tl = b * ST_B + et
                                nc.tensor.matmul(
                                    out=psum_t[:, eh * D:(eh + 1) * D],
                                    lhsT=sel_sb[:, tl * W:(tl + 1) * W],
                                    rhs=ets[b][:, et * D:(et + 1) * D],
                                    start=(b == 0 and dt == 0),
                                    stop=(b == NB - 1 and dt == ET_BK - 1),
                                )
                        if eh == PSUM_EP - 1:
                            er = (e - (PSUM_EP - 1)) % OUT_RING_EP
                            if er == 0:
                                out_ring = outp.tile(
                                    [W, OUT_RING_EP * D], mybir.dt.int16,
                                    tag="or", name=f"or{rep}_{e}")
                                ring_e0 = e - (PSUM_EP - 1)
                                flushed = 0
                            nc.scalar.activation(
                                out=out_ring[:, er * D:(er + PSUM_EP) * D],
                                in_=psum_t[:],
                                func=mybir.ActivationFunctionType.Copy,
                                scale=OSCALE,
                            )
                            pend = er + PSUM_EP
                            if er == OUT_RING_EP - PSUM_EP:
                                nc.sync.dma_start(
                                    out=oslots[:, (ring_e0 + flushed) * D:
                                                (ring_e0 + OUT_RING_EP) * D],
                                    in_=out_ring[:, flushed * D:
                                                 OUT_RING_EP * D],
                                )
                                pend = 0
                                flushed = 0
                            elif ring_e0 >= TAIL_E0 and pend - flushed >= 8:
                                nc.sync.dma_start(
                                    out=oslots[:, (ring_e0 + flushed) * D:
                                                (ring_e0 + pend) * D],
                                    in_=out_ring[:, flushed * D:pend * D],
                                )
                                flushed = pend
                if pend > flushed:
                    nc.sync.dma_start(
                        out=oslots[:, (ring_e0 + flushed) * D:
                                    (ring_e0 + pend) * D],
                        in_=out_ring[:, flushed * D:pend * D],
                    )
    _split_waits(nc, mybir)
    library_overlay.lower_extended_insts(nc)
    return nc


def _prep_core(values_c, seg_c, chunks, nidx, icol_off, NTT):
    """Slot layout for one table. Returns idxs [16, ICOL] int16 (unpadded:
    ceil(nidx/16) columns per (chunk, bucket)), cnts [nch*NB] int32,
    slot_seg [NTT*P] int64 (-1 for pad slots)."""
    ICOL = icol_off[-1]
    idxs = np.zeros((16, ICOL), np.int16)
    cnts = np.zeros(len(chunks) * NB, np.int32)
    slot_seg = np.full(NTT * P, -1, np.int64)
    bucket = values_c // BROWS
    for kc, ch in enumerate(chunks):
        S = ch["S"]
        vpos = np.arange(ch["pos0"], ch["pos0"] + ch["npos"])
        vb = bucket[vpos]
        for b in range(NB):
            i = kc * NB + b
            ni = nidx[i]
            pos_b = vpos[vb == b]           # position order preserved
            n = len(pos_b)
            assert n <= ni <= S, f"bucket overflow {n} > {ni} > {S}"
            rel = np.full(ni, -1, np.int16)
            rel[:n] = (values_c[pos_b] - b * BROWS).astype(np.int16)
            if n == 0:
                rel[0] = 0
                n = 1
            cnts[i] = n
            ic = icol_off[i]
            idxs[:, ic:ic + ni // 16] = rel.reshape(ni // 16, 16).T
            s0 = (ch["tile0"] + b * ch["ST_B"]) * P
            slot_seg[s0:s0 + len(pos_b)] = seg_c[pos_b]
    return np.ascontiguousarray(idxs), cnts, slot_seg


def _prepare(values, offsets, weights):
    """Host-side layout: returns (in_maps, meta) for the bass kernel."""
    values = np.asarray(values)
    offsets = np.asarray(offsets)
    weights = np.ascontiguousarray(np.asarray(weights, dtype=np.float32))
    wts = weights.astype(ml_dtypes.bfloat16)

    pos = np.arange(L)
    seg = np.empty((T, L), dtype=np.int64)
    for c in range(T):
        seg[c] = np.searchsorted(offsets[c, 1:], pos, side="right")

    # padded slots per (chunk, bucket): multiple of 512 covering max count
    bucket = values // BROWS
    S_list = []
    pos0 = 0
    for npos in CHUNK_NPOS:
        mx = 0
        for c in range(T):
            bc = np.bincount(bucket[c, pos0:pos0 + npos], minlength=NB)
            mx = max(mx, int(bc.max()))
        S_list.append(((mx + 511) // 512) * 512)
        pos0 += npos
    S_list = tuple(S_list)

    chunks, ICOL, NTT = _chunk_spec(S_list)
    NEPO = NTT // (NB * ET_BK)

    # per-(chunk, bucket) gather num_idxs: max count over cores, 16-rounded;
    # cmin: min count over cores (memset lower bound)
    nidx = []
    cmin = []
    for kc, ch in enumerate(chunks):
        for b in range(NB):
            mx = 1
            mn = ch["S"]
            for c in range(T):
                vb = bucket[c, ch["pos0"]:ch["pos0"] + ch["npos"]]
                n = int(np.count_nonzero(vb == b))
                mx = max(mx, n)
                mn = min(mn, max(n, 1))
            nidx.append(min(((mx + 15) // 16) * 16, ch["S"]))
            cmin.append(mn)
    nidx = tuple(nidx)
    cmin = tuple(cmin)
    icol_off = [0]
    for v in nidx:
        icol_off.append(icol_off[-1] + v // 16)

    prep = [_prep_core(values[c], seg[c], chunks, nidx, icol_off, NTT)
            for c in range(T)]

    # epoch windows: epoch e = tiles (2u, 2u+1) of each bucket
    # (cross-bucket grouping); W = max bag span
    slot_seg = np.stack([p[2] for p in prep])          # [T, NTT*P]
    ss = np.empty((T, NEPO, NB * ET_BK * P), np.int64)
    for ch in chunks:
        ST_B, NT = ch["ST_B"], ch["NT"]
        ne = ST_B // ET_BK
        ep0 = ch["tile0"] // (NB * ET_BK)
        blk = slot_seg[:, ch["tile0"] * P:(ch["tile0"] + NT) * P]
        blk = blk.reshape(T, NB, ne, ET_BK * P).transpose(0, 2, 1, 3)
        ss[:, ep0:ep0 + ne, :] = blk.reshape(T, ne, NB * ET_BK * P)
    ssm = np.ma.masked_equal(ss, -1)
    lo = ssm.min(axis=2).filled(0).astype(np.int64)    # [T, NEPO]
    hi = ssm.max(axis=2).filled(-1).astype(np.int64)
    span = np.maximum(hi - lo + 1, 0)
    W = int(span.max())
    W = max(4, (W + 3) // 4 * 4)
    assert W <= 128, f"epoch bag-window {W} exceeds PSUM partition limit"

    iota_np = np.tile(np.arange(W, dtype=np.int8), (P, 1))

    IC0 = icol_off[NB]
    in_maps = []
    for c in range(T):
        idxs_c, cnts_c, sseg_c = prep[c]
        # bag-id byte per slot: seg - lo(its cross-bucket epoch),
        # pad slots -> -1. Tile b*ST_B+et belongs to epoch ep0+et.
        lo_per_tile = np.empty(NTT, np.int64)
        for ch in chunks:
            ST_B = ch["ST_B"]
            ne = ST_B // ET_BK
            ep0 = ch["tile0"] // (NB * ET_BK)
            lo_per_tile[ch["tile0"]:ch["tile0"] + ch["NT"]] = np.tile(
                np.repeat(lo[c, ep0:ep0 + ne], ET_BK), NB)
        ep_lo = np.repeat(lo_per_tile, P)
        bid = np.where(sseg_c >= 0, sseg_c - ep_lo, -1).astype(np.int8)
        bid_tile = np.ascontiguousarray(bid.reshape(NTT, P).T)  # [P, NTT]
        in_maps.append({
            "wt": wts[c],
            "idxs0": np.ascontiguousarray(np.tile(idxs_c[:, :NB * SC0], (8, 1))),
            "idxsr": np.ascontiguousarray(np.tile(idxs_c[:, NB * SC0:], (8, 1))),
            "cnts": cnts_c.reshape(1, -1),
            "bagid": bid_tile,
            "iota": iota_np,
        })

    meta = dict(W=W, S_list=S_list, nidx=nidx, cmin=cmin, NEPO=NEPO, lo=lo,
                span=span)
    return in_maps, meta


def kernel(values, offsets, weights):
    from concourse.bass_utils import run_bass_kernel_spmd

    in_maps, meta = _prepare(values, offsets, weights)
    W, NEPO, lo, span = meta["W"], meta["NEPO"], meta["lo"], meta["span"]

    key = (W, meta["S_list"], meta["nidx"], meta["cmin"])
    if key not in _compiled:
        _compiled.clear()
        _compiled[key] = _build(*key)
    nc = _compiled[key]

    global _last_inmaps
    _last_inmaps = in_maps
    res = run_bass_kernel_spmd(nc, in_maps, core_ids=list(range(T)))

    out = np.zeros((B, T * D), dtype=np.float32)
    for c in range(T):
        osl = np.asarray(res.results[c]["oslots"], dtype=np.float32).reshape(W, NEPO, D) * (1.0 / OSCALE)
        pooled = np.zeros((B, D), dtype=np.float32)
        for e in range(NEPO):
            n = int(span[c, e])
            if n == 0:
                continue
            lo_e = int(lo[c, e])
            pooled[lo_e:lo_e + n] += osl[:n, e, :]
        out[:, c * D:(c + 1) * D] = pooled
    return out


if __name__ == "__main__":
    rng = np.random.default_rng(0)
    values = rng.integers(0, V, size=(T, L)).astype(np.int64)
    inner = np.sort(rng.integers(0, L, size=(T, B - 1)), axis=1)
    offsets = np.concatenate(
        [np.zeros((T, 1), np.int64), inner, np.full((T, 1), L, np.int64)], axis=1
    )
    weights = (rng.standard_normal((T, V, D)) * 0.01).astype(np.float32)
    out = kernel(values, offsets, weights)
    exp = np.zeros((B, T * D), dtype=np.float32)
    for c in range(T):
        pooled = np.zeros((B, D), np.float32)
        np.add.at(pooled, np.searchsorted(offsets[c, 1:], np.arange(L), side="right"), weights[c][values[c]])
        exp[:, c * D:(c + 1) * D] = pooled
    err = np.linalg.norm(out - exp) / np.linalg.norm(exp)
    print("self-check rel err:", err)


# revision 44
# speedup vs baseline: 25.2809x; 25.2809x over previous
"""GroupedEmbeddingBag kernel for 8 trn2 NeuronCores.

Table-parallel: core c handles table c (weights[c], values[c], offsets[c]).

Per core the id stream is split into position-chunks (14 big + a pyramid of
shrinking tail chunks, so the post-gather pipeline drain at the end is
short); within a chunk ids are bucket-sorted into 4 contiguous table-row
ranges of 25000 rows so that dma_gather (InstDMAGatherAnt, int16 relative
indices) can pull thousands of rows per SWDGE instruction — the
per-instruction descriptor-generation overhead that dominated an
indirect_dma_start-per-tile design is amortized away. Gathered rows land in
SBUF in (chunk, bucket, position) slot order; the host knows the exact slot
layout, so it ships a per-slot bag-id byte (epoch-relative). The DVE
expands bag-ids into one-hot bf16 selection matrices on-chip (is_equal vs
an iota constant), TensorE matmuls segment-sum epochs into PSUM bag
windows (an epoch = tile t from EACH of the 4 buckets, which cover the same
~512 positions, so the window W is ~40 instead of ~68; two epochs share one
psum tile), the Scalar engine copies finished pairs to an SBUF ring with an
int16 output scale, and 48-epoch ring segments stream back to DRAM. The host overlap-adds the epoch windows into
the final pooled output and divides the scale back out.

Pipeline-keeping details (the kernel is DMA-descriptor-bound: ~205k row
descriptors/core at 512 effective bytes each):
- Each bucket of a chunk gathers into its own et tile, so a gather only
  waits on the matmuls of its own bucket from EP_BUFS chunks back, and
  matmuls start as soon as their bucket's transfer lands.
- Chunk 0's idx columns are a separate tiny DRAM param, so the first
  gather's descriptor generation starts ~1us in.
- DVE memsets of the first et ring generation run concurrently with the
  first transfers (pad slots must stay finite since 0*NaN poisons PSUM);
  after that, buffers only ever hold stale gathered rows.

Weights are bf16 (PSUM accumulation fp32): rel-err vs the fp32 reference
~2e-3, far inside the 2e-2 gate.
"""

import os
import sys

sys.path.insert(0, "/opt/trn_rl_repo")

import numpy as np
import ml_dtypes

T, V, D, B = 8, 100000, 128, 4096
L = 204800
P = 128
NB = 4                      # index-range buckets (int16 addressing limit)
BROWS = V // NB             # 25000 rows per bucket
# 14 big chunks + a pyramid tail so the post-gather drain is short and each
# tail chunk's et-ring dependency (EP_BUFS back) is already drained.
CHUNK_NPOS = [12800] * 14 + [9600, 6400, 4800, 3200, 1600]
EP_T = 4                    # slot-tiles per matmul group (one per bucket)
ET_BK = 2                   # tiles per bucket per epoch (epoch = NB*ET_BK tiles)
OUT_RING_EP = 48             # epochs per output DMA
EP_BUFS = 4                 # et ring depth; first EP_BUFS chunks gather-fill
SEL_BUFS = 3

_compiled = {}


def _chunk_spec(S_list):
    """Per-chunk layout: slots per (chunk, bucket) are padded to S_k
    (multiple of 512). Returns list of dicts + totals."""
    chunks = []
    pos = 0
    icol = 0
    tile0 = 0
    for npos, S_k in zip(CHUNK_NPOS, S_list, strict=True):
        assert S_k % 512 == 0
        chunks.append(dict(
            pos0=pos, npos=npos, S=S_k, SC=S_k // 16, ST_B=S_k // P,
            NT=NB * (S_k // P), icol0=icol, tile0=tile0,
        ))
        pos += npos
        icol += NB * (S_k // 16)
        tile0 += NB * (S_k // P)
    assert pos == L
    return chunks, icol, tile0


def _patch_drain(tile_mod, mybir):
    from concourse.vector_clock import ScopedClock

    def _patched(self, tick_clock, wait_clock):
        # this walrus build allows only ONE sync-wait on the tail Drain:
        # spread the rest over preceding nops, one wait each. Rotate the
        # nops across all engine sequencers so their dispatch (~25-50ns
        # each) runs in parallel instead of serializing ~64 nops on SP.
        NNOPS = 64
        engs = [self.nc.sync, self.nc.gpsimd, self.nc.vector,
                self.nc.scalar, self.nc.tensor]
        nops = [engs[i % len(engs)].nop(nofuse=True, hint=f"dw_{i}")
                for i in range(NNOPS)]
        drain_inst = self.nc.sync.drain()
        wait_clock.add_sem_waits(
            drain_inst.ins, ScopedClock({None: tick_clock.global_clock})
        )
        dsi = drain_inst.ins.sync_info
        waits = list(dsi.on_wait) if dsi else []
        if len(waits) > 1:
            del dsi.on_wait[1:]
            rest = waits[1:]
            assert len(rest) <= NNOPS, f"too many drain waits: {len(waits)}"
            for nop, w in zip(nops, rest):
                nsi = nop.ins.sync_info
                if nsi is None:
                    nop.ins.sync_info = mybir.SyncInfo(on_wait=[w], on_update=[])
                else:
                    nsi.on_wait.append(w)
        self.nc.all_engine_barrier()
        popped = self.nc._tile_sem_poison_stack.pop()
        assert popped is self._sem_poison
        self.nc.clear_and_free_semaphores(list(self.sems.allocated().values()))
        self.nc.all_engine_barrier()

    tile_mod.TileContext._drain_and_barrier = _patched


def _split_waits(nc, mybir, maxw=1):
    # this walrus build rejects >1 sync-wait on an instruction: hoist extra
    # waits onto same-engine nops spliced in directly before it.
    cnt = 0
    for fn in nc.m.functions:
        for blk in fn.blocks:
            new_insts = []
            for inst in blk.instructions:
                si = inst.sync_info
                if si is not None and len(si.on_wait) > maxw:
                    extra = list(si.on_wait[maxw:])
                    del si.on_wait[maxw:]
                    for w in extra:
                        nop = mybir.InstNoOp(
                            name=f"waitnop-{cnt}", engine=inst.engine, ins=[], outs=[]
                        )
                        cnt += 1
                        nop.sync_info = mybir.SyncInfo(on_wait=[w], on_update=[])
                        new_insts.append(nop)
                new_insts.append(inst)
            blk.instructions[:] = new_insts
    return cnt


def _build(W, S_list, nidx, cmin, reps=1):
    """W: epoch bag-window (PSUM partitions). S_list: per-chunk padded slots
    per bucket. nidx: per-(chunk,bucket) gather num_idxs (max count over
    cores, 16-rounded) — the idx arrays carry no S-padding. reps>1 repeats
    the whole body for timing-calibration builds."""
    import concourse.bass as bass
    import concourse.mybir as mybir
    import concourse.tile as tile
    from concourse import library_config, library_overlay

    _patch_drain(tile, mybir)

    chunks, ICOL, NTT = _chunk_spec(S_list)
    NEPO = NTT // (NB * ET_BK)
    # Shared idx column ranges: all 4 buckets of chunk kc point at the same
    # columns; bucket b's list lives only in partition band [32b, 32b+32)
    # (the gather ucode's queue-b cpu pair reads just that band), so the
    # payload ships 2 copies instead of 8.
    coff = [0]
    for kc in range(len(chunks)):
        coff.append(coff[-1] + max(nidx[kc * NB + b] for b in range(NB)) // 16)
    QC0 = coff[1]
    QCR = coff[-1] - coff[1]

    bf16 = mybir.dt.bfloat16

    nc = bass.Bass(num_swdge_queues=4)
    wt = nc.declare_dram_parameter("wt", [V, D], bf16, isOutput=False)
    # idx payload host-replicated to the 8 Q7 core groups ([128, ...]) so each
    # lands in one DMA. Chunk 0's columns are split out so the first gathers
    # only wait on a small load.
    idxs0 = nc.declare_dram_parameter("idxs0", [P, NB * SC0], mybir.dt.int16, isOutput=False)
    idxsr = nc.declare_dram_parameter("idxsr", [P, ICOL - NB * SC0], mybir.dt.int16, isOutput=False)
    cnts = nc.declare_dram_parameter("cnts", [1, len(chunks) * NB], mybir.dt.int32, isOutput=False)
    bagid = nc.declare_dram_parameter("bagid", [P, NTT], mybir.dt.int8, isOutput=False)
    iota = nc.declare_dram_parameter("iota", [P, W], mybir.dt.int8, isOutput=False)
    oslots = nc.declare_dram_parameter("oslots", [W, NEPO * D], mybir.dt.int16, isOutput=True)

    with tile.TileContext(nc) as tc:
        nc.gpsimd.load_library(library_config.mlp)
        with (
            tc.tile_pool(name="inp", bufs=1) as inp,
            tc.tile_pool(name="selp", bufs=SEL_BUFS) as selp,
            tc.tile_pool(name="ep", bufs=EP_BUFS) as ep,
            tc.tile_pool(name="outp", bufs=2) as outp,
            tc.tile_pool(name="psum", bufs=8, space="PSUM") as psump,
        ):
            cnts_sb = inp.tile([1, len(chunks) * NB], mybir.dt.int32)
            nc.sync.dma_start(out=cnts_sb[:], in_=cnts[:])
            idxs0_sb = inp.tile([P, NB * SC0], mybir.dt.int16)
            nc.sync.dma_start(out=idxs0_sb[:], in_=idxs0[:])
            iota_sb = inp.tile([P, W], mybir.dt.int8)
            nc.sync.dma_start(out=iota_sb[:], in_=iota[:])
            bagid_sb = inp.tile([P, NTT], mybir.dt.int8)
            nc.sync.dma_start(out=bagid_sb[:], in_=bagid[:])
            idxsr_sb = inp.tile([P, ICOL - NB * SC0], mybir.dt.int16)
            nc.sync.dma_start(out=idxsr_sb[:], in_=idxsr[:])

            cregs = [nc.gpsimd.alloc_register(name=f"creg{b}") for b in range(NB)]

            init_tiles = []

            out_ring = None
            psum_t = None
            for rep in range(reps):
                pend = 0      # epochs pending in out_ring
                flushed = 0   # epochs of this ring already flushed
                ring_e0 = 0
                TAIL_E0 = NEPO - (NEPO % OUT_RING_EP)  # first epoch of the
                # final partial ring: flush it in 8-epoch slices so the tail
                # output overlaps the tail compute
                for kc, ch in enumerate(chunks):
                    S, SC, ST_B, NT = ch["S"], ch["SC"], ch["ST_B"], ch["NT"]
                    ets = [
                        ep.tile([P, ST_B * D], bf16, tag=f"e{b}",
                                name=f"et{rep}_{kc}_{b}")
                        for b in range(NB)
                    ]
                    if rep == 0 and kc < EP_BUFS:
                        # ring buffers start with arbitrary SBUF bits; pad
                        # slots (idx -1 -> no DMA write) must stay finite
                        # since 0*NaN poisons PSUM. Only the tail beyond the
                        # min gather count over cores can stay unwritten —
                        # memset just that slice (12x smaller, keeps DVE off
                        # the gather critical path).
                        for b in range(NB):
                            mstart = (cmin[kc * NB + b] // P) * D
                            if mstart < ST_B * D:
                                nc.vector.memset(ets[b][:, mstart:], 0)
                    for b in range(NB):
                        i = kc * NB + b
                        ni = nidx[i]
                        nc_cols = ni // 16
                        if kc == 0:
                            idx_ap = idxs0_sb[:, coff[0]:coff[0] + nc_cols]
                        else:
                            j = coff[kc] - coff[1]
                            idx_ap = idxsr_sb[:, j:j + nc_cols]
                        ntile = (ni + P - 1) // P
                        nc.gpsimd.load(cregs[b], cnts_sb[:1, i:i + 1])
                        nc.gpsimd.dma_gather(
                            ets[b][:, :ntile * D].rearrange(
                                "p (s d) -> p s d", d=D),
                            wt[b * BROWS:(b + 1) * BROWS, :],
                            idx_ap,
                            ni,
                            cregs[b],
                            D,
                            single_packet=False,
                            queue_num=b,
                        )
                    sel_sb = selp.tile([P, NT * W], bf16, tag="sel",
                                       name=f"sel{rep}_{kc}")
                    nc.vector.tensor_tensor(
                        out=sel_sb[:].rearrange("p (t w) -> p t w", w=W),
                        in0=bagid_sb[:, ch["tile0"]:ch["tile0"] + NT]
                        .rearrange("p t -> p t ()")
                        .to_broadcast([P, NT, W]),
                        in1=iota_sb[:]
                        .rearrange("p w -> p () w")
                        .to_broadcast([P, NT, W]),
                        op=mybir.AluOpType.is_equal,
                    )
                    ep0 = ch["tile0"] // (NB * ET_BK)
                    for u in range(ST_B // ET_BK):
                        # epoch = tiles (2u, 2u+1) from EACH bucket: they
                        # cover the same ~ET_BK*EP_T*P positions, keeping
                        # the bag window W small (~48)
                        e = ep0 + u
                        eh = e % PSUM_EP    # epoch slot within the psum tile
                        if eh == 0:
                            psum_t = psump.tile(
                                [W, PSUM_EP * D], mybir.dt.float32, tag="ps")
                        for b in range(NB):
                            for dt in range(ET_BK):
                                et = u * ET_BK + dt
                                tl = b * ST_B + et
                                nc.tensor.matmul(
                                    out=psum_t[:, eh * D:(eh + 1) * D],
                                    lhsT=sel_sb[:, tl * W:(tl + 1) * W],
                                    rhs=ets[b][:, et * D:(et + 1) * D],
                                    start=(b == 0 and dt == 0),
                                    stop=(b == NB - 1 and dt == ET_BK - 1),
                                )
                        if eh == PSUM_EP - 1:
                            er = (e - (PSUM_EP - 1)) % OUT_RING_EP
                            if er == 0:
                                out_ring = outp.tile(
                                    [W, OUT_RING_EP * D], mybir.dt.int16,
                                    tag="or", name=f"or{rep}_{e}")
                                ring_e0 = e - (PSUM_EP - 1)
                                flushed = 0
                            nc.scalar.activation(
                                out=out_ring[:, er * D:(er + PSUM_EP) * D],
                                in_=psum_t[:],
                                func=mybir.ActivationFunctionType.Copy,
                                scale=OSCALE,
                            )
                            pend = er + PSUM_EP
                            if er == OUT_RING_EP - PSUM_EP:
                                nc.sync.dma_start(
                                    out=oslots[:, (ring_e0 + flushed) * D:
                                                (ring_e0 + OUT_RING_EP) * D],
                                    in_=out_ring[:, flushed * D:
                                                 OUT_RING_EP * D],
                                )
                                pend = 0
                                flushed = 0
                            elif ring_e0 >= TAIL_E0 and pend - flushed >= 8:
                                nc.sync.dma_start(
                                    out=oslots[:, (ring_e0 + flushed) * D:
                                                (ring_e0 + pend) * D],
                                    in_=out_ring[:, flushed * D:pend * D],
                                )
                                flushed = pend
                if pend > flushed:
                    nc.sync.dma_start(
                        out=oslots[:, (ring_e0 + flushed) * D:
                                    (ring_e0 + pend) * D],
                        in_=out_ring[:, flushed * D:pend * D],
                    )
    _split_waits(nc, mybir)
    library_overlay.lower_extended_insts(nc)
    return nc


def _prep_core(values_c, seg_c, chunks, nidx, coff, QC0, QCR, NTT):
    """Slot layout for one table. Returns idxs0 [128, QC0] / idxsr
    [128, QCR] int16: chunk kc's buckets share column range
    [coff[kc], coff[kc+1]); bucket b's list sits in partition band
    [32b, 32b+32) as two 16-row copies (the ucode's queue-b cpus read only
    that band). Plus cnts [nch*NB] int32 and slot_seg [NTT*P] int64."""
    idxs0 = np.zeros((P, QC0), np.int16)
    idxsr = np.zeros((P, QCR), np.int16)
    cnts = np.zeros(len(chunks) * NB, np.int32)
    slot_seg = np.full(NTT * P, -1, np.int64)
    bucket = values_c // BROWS
    for kc, ch in enumerate(chunks):
        S = ch["S"]
        vpos = np.arange(ch["pos0"], ch["pos0"] + ch["npos"])
        vb = bucket[vpos]
        for b in range(NB):
            i = kc * NB + b
            ni = nidx[i]
            pos_b = vpos[vb == b]           # position order preserved
            n = len(pos_b)
            assert n <= ni <= S, f"bucket overflow {n} > {ni} > {S}"
            rel = np.full(ni, -1, np.int16)
            rel[:n] = (values_c[pos_b] - b * BROWS).astype(np.int16)
            if n == 0:
                rel[0] = 0
                n = 1
            cnts[i] = n
            blk = rel.reshape(ni // 16, 16).T
            if kc == 0:
                dst, jc = idxs0, coff[0]
            else:
                dst, jc = idxsr, coff[kc] - coff[1]
            for r in (32 * b, 32 * b + 16):
                dst[r:r + 16, jc:jc + ni // 16] = blk
            s0 = (ch["tile0"] + b * ch["ST_B"]) * P
            slot_seg[s0:s0 + len(pos_b)] = seg_c[pos_b]
    return idxs0, idxsr, cnts, slot_seg


def _prepare(values, offsets, weights):
    """Host-side layout: returns (in_maps, meta) for the bass kernel."""
    values = np.asarray(values)
    offsets = np.asarray(offsets)
    weights = np.ascontiguousarray(np.asarray(weights, dtype=np.float32))
    wts = weights.astype(ml_dtypes.bfloat16)

    pos = np.arange(L)
    seg = np.empty((T, L), dtype=np.int64)
    for c in range(T):
        seg[c] = np.searchsorted(offsets[c, 1:], pos, side="right")

    # padded slots per (chunk, bucket): multiple of 512 covering max count
    bucket = values // BROWS
    S_list = []
    pos0 = 0
    for npos in CHUNK_NPOS:
        mx = 0
        for c in range(T):
            bc = np.bincount(bucket[c, pos0:pos0 + npos], minlength=NB)
            mx = max(mx, int(bc.max()))
        S_list.append(((mx + 511) // 512) * 512)
        pos0 += npos
    S_list = tuple(S_list)

    chunks, ICOL, NTT = _chunk_spec(S_list)
    NEPO = NTT // (NB * ET_BK)

    # per-(chunk, bucket) gather num_idxs: max count over cores, 16-rounded;
    # cmin: min count over cores (memset lower bound)
    nidx = []
    cmin = []
    for kc, ch in enumerate(chunks):
        for b in range(NB):
            mx = 1
            mn = ch["S"]
            for c in range(T):
                vb = bucket[c, ch["pos0"]:ch["pos0"] + ch["npos"]]
                n = int(np.count_nonzero(vb == b))
                mx = max(mx, n)
                mn = min(mn, max(n, 1))
            nidx.append(min(((mx + 15) // 16) * 16, ch["S"]))
            cmin.append(mn)
    nidx = tuple(nidx)
    cmin = tuple(cmin)
    # shared column offsets: all 4 buckets of chunk kc use the same column
    # range; bucket b's idx list lives in partition band [32b, 32b+32)
    # (two 16-row copies for the queue's two Q7 cpus)
    coff = [0]
    for kc in range(len(chunks)):
        coff.append(coff[-1] + max(nidx[kc * NB + b] for b in range(NB)) // 16)
    QC0 = coff[1]
    QCR = coff[-1] - coff[1]

    prep = [_prep_core(values[c], seg[c], chunks, nidx, coff, QC0, QCR, NTT)
            for c in range(T)]

    # epoch windows: epoch e = tiles (2u, 2u+1) of each bucket
    # (cross-bucket grouping); W = max bag span
    slot_seg = np.stack([p[3] for p in prep])          # [T, NTT*P]
    ss = np.empty((T, NEPO, NB * ET_BK * P), np.int64)
    for ch in chunks:
        ST_B, NT = ch["ST_B"], ch["NT"]
        ne = ST_B // ET_BK
        ep0 = ch["tile0"] // (NB * ET_BK)
        blk = slot_seg[:, ch["tile0"] * P:(ch["tile0"] + NT) * P]
        blk = blk.reshape(T, NB, ne, ET_BK * P).transpose(0, 2, 1, 3)
        ss[:, ep0:ep0 + ne, :] = blk.reshape(T, ne, NB * ET_BK * P)
    ssm = np.ma.masked_equal(ss, -1)
    lo = ssm.min(axis=2).filled(0).astype(np.int64)    # [T, NEPO]
    hi = ssm.max(axis=2).filled(-1).astype(np.int64)
    span = np.maximum(hi - lo + 1, 0)
    W = int(span.max())
    W = max(4, (W + 3) // 4 * 4)
    assert W <= 128, f"epoch bag-window {W} exceeds PSUM partition limit"

    iota_np = np.tile(np.arange(W, dtype=np.int8), (P, 1))

    in_maps = []
    for c in range(T):
        idxs0_c, idxsr_c, cnts_c, sseg_c = prep[c]
        # bag-id byte per slot: seg - lo(its cross-bucket epoch),
        # pad slots -> -1. Tile b*ST_B+et belongs to epoch ep0+et.
        lo_per_tile = np.empty(NTT, np.int64)
        for ch in chunks:
            ST_B = ch["ST_B"]
            ne = ST_B // ET_BK
            ep0 = ch["tile0"] // (NB * ET_BK)
            lo_per_tile[ch["tile0"]:ch["tile0"] + ch["NT"]] = np.tile(
                np.repeat(lo[c, ep0:ep0 + ne], ET_BK), NB)
        ep_lo = np.repeat(lo_per_tile, P)
        bid = np.where(sseg_c >= 0, sseg_c - ep_lo, -1).astype(np.int8)
        bid_tile = np.ascontiguousarray(bid.reshape(NTT, P).T)  # [P, NTT]
        in_maps.append({
            "wt": wts[c],
            "idxs0": np.ascontiguousarray(np.tile(idxs_c[:, :NB * SC0], (8, 1))),
            "idxsr": np.ascontiguousarray(np.tile(idxs_c[:, NB * SC0:], (8, 1))),
            "cnts": cnts_c.reshape(1, -1),
            "bagid": bid_tile,
            "iota": iota_np,
        })

    meta = dict(W=W, S_list=S_list, nidx=nidx, cmin=cmin, NEPO=NEPO, lo=lo,
                span=span)
    return in_maps, meta


def kernel(values, offsets, weights):
    from concourse.bass_utils import run_bass_kernel_spmd

    in_maps, meta = _prepare(values, offsets, weights)
    W, NEPO, lo, span = meta["W"], meta["NEPO"], meta["lo"], meta["span"]

    key = (W, meta["S_list"], meta["nidx"], meta["cmin"])
    if key not in _compiled:
        _compiled.clear()
        _compiled[key] = _build(*key)
    nc = _compiled[key]

    global _last_inmaps
    _last_inmaps = in_maps
    res = run_bass_kernel_spmd(nc, in_maps, core_ids=list(range(T)))

    out = np.zeros((B, T * D), dtype=np.float32)
    for c in range(T):
        osl = np.asarray(res.results[c]["oslots"], dtype=np.float32).reshape(W, NEPO, D) * (1.0 / OSCALE)
        pooled = np.zeros((B, D), dtype=np.float32)
        for e in range(NEPO):
            n = int(span[c, e])
            if n == 0:
                continue
            lo_e = int(lo[c, e])
            pooled[lo_e:lo_e + n] += osl[:n, e, :]
        out[:, c * D:(c + 1) * D] = pooled
    return out


if __name__ == "__main__":
    rng = np.random.default_rng(0)
    values = rng.integers(0, V, size=(T, L)).astype(np.int64)
    inner = np.sort(rng.integers(0, L, size=(T, B - 1)), axis=1)
    offsets = np.concatenate(
        [np.zeros((T, 1), np.int64), inner, np.full((T, 1), L, np.int64)], axis=1
    )
    weights = (rng.standard_normal((T, V, D)) * 0.01).astype(np.float32)
    out = kernel(values, offsets, weights)
    exp = np.zeros((B, T * D), dtype=np.float32)
    for c in range(T):
        pooled = np.zeros((B, D), np.float32)
        np.add.at(pooled, np.searchsorted(offsets[c, 1:], np.arange(L), side="right"), weights[c][values[c]])
        exp[:, c * D:(c + 1) * D] = pooled
    err = np.linalg.norm(out - exp) / np.linalg.norm(exp)
    print("self-check rel err:", err)
